# revision 78
# baseline (speedup 1.0000x reference)
"""Transformer block kernel for TRN2 (Bass/Tile), one batch element per core.

Computes (per core, x [1024, 768] f32):
    h  = LN(x) (gamma/beta pre-folded into weights on host), stored fp8
    qk = h @ qkw + qkb (fp8 DoubleRow, K=256/mm); q,k fp8 head-major
    v  = h @ vw (fp8 DR), packed into vext with a ones column (bf16)
    S^T[m,n] = (k_m . q_n) / 8 for two heads concurrently (row-split PE)
    P = exp(S^T) bf16; oe = [v;1]^T @ P -> rows 0..63 o^T, row 64 denom
    o^T = oe / denom -> fp8;  x1 = x + o @ pw + pb (fp8 DR)
    h2 = LN2(x1) -> fp8; out = x1 + gelu(h2 @ f1w + f1b) @ f2w + f2b (fp8 DR)

Two-half pipeline: attention exp for tokens 512..1023 (ACT engine) overlaps
the fc2 matmuls for tokens 0..511 (PE), since exp is the attention bottleneck.

fp8 weights are pre-scaled by powers of 2 on host; the inverse scale is folded
into the PSUM->SBUF copy (DVE tensor_scalar or ACT activation scale).
"""

import sys
from contextlib import ExitStack

if "/opt/trn_rl_repo" not in sys.path:
    sys.path.insert(0, "/opt/trn_rl_repo")

import numpy as np

import concourse.bass as bass
import concourse.mybir as mybir
from concourse.masks import make_identity

F32 = mybir.dt.float32
BF16 = mybir.dt.bfloat16
F8 = mybir.dt.float8e4
DRM = mybir.MatmulPerfMode.DoubleRow
AF = mybir.ActivationFunctionType
ALU = mybir.AluOpType

P = 128
EMB = 768
SEQ = 1024
NH = 12
HD = 64
MLPD = 3072
EC = EMB // P      # 6 embedding chunks
ECP = EC // 2      # 3 DR k-pair chunks
NT = SEQ // P      # 8 token tiles
NC2 = SEQ // 512   # 2 token halves
HC = MLPD // P     # 24 hidden chunks
HCP = HC // 2      # 12 hidden pairs
HP = NH // 2       # 6 head pairs
EPS = 1e-5
SCALE = HD ** -0.5

# fp8 weight pre-scales (powers of two; inverse folded into PSUM copy-out)
QQK = 4096.0
QV = 4096.0
QP = 4096.0
QF1 = 4096.0
QF2 = 8192.0


def _ln_stats(nc, x_ap, mv, stats, eps_t):
    """bn stats + rstd for one [128, EMB] tile; mv = [mean, rstd]."""
    xg = x_ap.rearrange("p (g d) -> p g d", d=256)
    for g in range(3):
        nc.vector.bn_stats(out=stats[:, g, :], in_=xg[:, g, :])
    nc.vector.bn_aggr(out=mv, in_=stats)
    nc.scalar.activation(out=mv[:, 1:2], in_=mv[:, 1:2], func=AF.Sqrt, bias=eps_t, scale=1.0)
    nc.vector.reciprocal(out=mv[:, 1:2], in_=mv[:, 1:2])


def _ln_apply(nc, x_ap, h_out, mv, negmr=None):
    if negmr is not None:
        # ACT variant for the head phase (DVE is congested, ACT idle):
        # h = x*rstd + (-mean*rstd)
        nc.scalar.activation(
            out=h_out, in_=x_ap, func=AF.Identity,
            scale=mv[:, 1:2], bias=negmr,
        )
    else:
        nc.vector.tensor_scalar(
            out=h_out,
            in0=x_ap,
            scalar1=mv[:, 0:1],
            scalar2=mv[:, 1:2],
            op0=ALU.subtract,
            op1=ALU.mult,
        )


def _transpose_to_featmajor(nc, tc, pool_ps, src_tok, dstT, t, tag="tr", copy_dve=False):
    """PE-transpose token-major src_tok [128, EMB] into dstT [:, e, t*128:(t+1)*128]."""
    ident = tc._block_ident
    for group_start, group_n in ((0, 4), (4, 2)):
        # [P, 8*P] bf16 = 2KB/partition so the tile is size-compatible with
        # the shared "pqk" psum tag; only the first 4*P columns are used.
        ptr = pool_ps.tile([P, 8 * P], BF16, tag=tag, bufs=2, name=f"ptr_t{t}_{group_start}_{dstT.tensor.name}")
        for j in range(group_n):
            e = group_start + j
            nc.tensor.transpose(
                ptr[:, j * P:(j + 1) * P],
                src_tok[:, e * P:(e + 1) * P],
                ident,
            )
        nc.scalar.copy(
            out=dstT[:, group_start:group_start + group_n, t * P:(t + 1) * P],
            in_=ptr[:, :group_n * P].rearrange("p (j q) -> p j q", q=P),
        )


def build_block(tc, outs, ins):
    nc = tc.nc
    x_d = ins["x"]
    qkw_d, qkb_d = ins["qkw"], ins["qkb"]
    vw_d = ins["vw"]
    pw_d, pb_d = ins["pw"], ins["pb"]
    f1w_d, f1b_d = ins["f1w"], ins["f1b"]
    f2w_d, f2b_d = ins["f2w"], ins["f2b"]
    out_d = outs["out"]

    with ExitStack() as ctx:
        consts = ctx.enter_context(tc.tile_pool(name="consts", bufs=1))
        ident = consts.tile([P, P], BF16)
        make_identity(nc, ident)
        tc._block_ident = ident
        eps_t = consts.tile([P, 1], F32)
        nc.vector.memset(eps_t, EPS)
        qkb_sb = consts.tile([P, 2 * EC], F32)
        pb_sb = consts.tile([P, EC], F32)
        f1b_sb = consts.tile([P, HC], F32)
        f2b_sb = consts.tile([P, EC], F32)

        # Persistent SBUF tensors
        glob = ctx.enter_context(tc.tile_pool(name="glob", bufs=1))
        x1 = glob.tile([P, NT, EMB], F32)             # residual stream
        actT = glob.tile([P, EC, SEQ], F8)            # LN1 out, feature-major
        h2T = glob.tile([P, EC, SEQ], F8)             # LN2 out, feature-major
        oT = glob.tile([P, EC, SEQ], F8)              # attention out, feature-major
        qkT = glob.tile([P, 2 * HP, SEQ], F8)         # q (0..5) and k (6..11) blocks
        vext = glob.tile([P, NT, NH, HD + 1], BF16)   # v + ones column
        a2g = glob.tile([P, HCP, 2, 512], F8)         # gelu out, hc-paired
        qkw_sb = glob.tile([P, EC, 2 * EMB], F8)
        vw_sb = glob.tile([P, EC, EMB], F8)
        pw_sb = glob.tile([P, EC, EMB], F8)
        f1w_sb = glob.tile([P, EC, MLPD], F8)
        f2w_sb = glob.tile([P, HC, EMB], F8)

        work = ctx.enter_context(tc.tile_pool(name="work", bufs=3))
        stat_pool = ctx.enter_context(tc.tile_pool(name="stat", bufs=4))

        # ---- DMAs: x first (blocks everything), then weights in use order ----
        # (spreading these across other engine queues was tried and measured
        # slower: scalar-queue copies delay the ACT stream, gpsimd DGE is slow)
        x_r = x_d.rearrange("(t p) e -> p t e", p=P)
        for t in range(NT):
            nc.sync.dma_start(out=x1[:, t, :], in_=x_r[:, t, :])
        nc.sync.dma_start(out=qkw_sb, in_=qkw_d.rearrange("(kc p) o -> p kc o", p=P))
        nc.sync.dma_start(out=vw_sb, in_=vw_d.rearrange("(kc p) o -> p kc o", p=P))
        nc.sync.dma_start(out=qkb_sb, in_=qkb_d.rearrange("(m p) -> p m", p=P))
        nc.sync.dma_start(out=pb_sb, in_=pb_d.rearrange("(m p) -> p m", p=P))
        nc.sync.dma_start(out=f1b_sb, in_=f1b_d.rearrange("(m p) -> p m", p=P))
        nc.sync.dma_start(out=f2b_sb, in_=f2b_d.rearrange("(m p) -> p m", p=P))
        nc.sync.dma_start(out=pw_sb, in_=pw_d.rearrange("(kc p) e -> p kc e", p=P))
        nc.sync.dma_start(out=f1w_sb, in_=f1w_d.rearrange("(kc p) o -> p kc o", p=P))
        nc.sync.dma_start(out=f2w_sb, in_=f2w_d.rearrange("(hc p) e -> p hc e", p=P))

        # ================= Phase A: LN1 + transpose to actT =================
        with tc.tile_pool(name="psA", space="PSUM", bufs=2) as psA:
            hs, mvs = [], []
            for t in range(NT):
                mv = stat_pool.tile([P, 2], F32, tag="mv", bufs=NT, name=f"mv1_{t}")
                stats = stat_pool.tile([P, 3, 6], F32, tag="stats", name=f"st1_{t}")
                _ln_stats(nc, x1[:, t, :], mv, stats, eps_t)
                mvs.append(mv)
            for t in range(NT):
                h_t = work.tile([P, EMB], BF16, tag="h", bufs=4, name=f"h_{t}")
                _ln_apply(nc, x1[:, t, :], h_t, mvs[t])
                hs.append(h_t)
            for t in range(NT):
                _transpose_to_featmajor(nc, tc, psA, hs[t], actT, t)

        # ---------- shared emitters ----------
        def emit_qk(ps, hp):
            """qk projection (DR) for head-pair hp -> qkT blocks hp and HP+hp."""
            for m in (hp, HP + hp):
                for nn in range(NC2):
                    pqk = ps.tile([P, 512], F32, tag="pqk", bufs=2, name=f"pqk_{m}_{nn}")
                    for i in range(ECP):
                        nc.tensor.matmul(
                            pqk,
                            qkw_sb[:, 2 * i:2 * i + 2, m * P:(m + 1) * P],
                            actT[:, 2 * i:2 * i + 2, nn * 512:(nn + 1) * 512],
                            start=(i == 0),
                            stop=(i == ECP - 1),
                            perf_mode=DRM,
                        )
                    nc.vector.tensor_scalar(
                        out=qkT[:, m, nn * 512:(nn + 1) * 512],
                        in0=pqk,
                        scalar1=1.0 / QQK,
                        scalar2=qkb_sb[:, m:m + 1],
                        op0=ALU.mult,
                        op1=ALU.add,
                    )

        def emit_v(ps, t):
            """v projection (DR) for token tile t -> vext[:, t, :, 0:HD]."""
            for half, (c0, cw, h0, hn) in ((0, (0, 512, 0, 8)), (1, (512, 256, 8, 4))):
                pv = ps.tile([P, 512], F32, tag="pqk", bufs=2, name=f"pv_{t}_{half}")
                for i in range(ECP):
                    nc.tensor.matmul(
                        pv[:, :cw],
                        actT[:, 2 * i:2 * i + 2, t * P:(t + 1) * P],
                        vw_sb[:, 2 * i:2 * i + 2, c0:c0 + cw],
                        start=(i == 0),
                        stop=(i == ECP - 1),
                        perf_mode=DRM,
                    )
                nc.vector.tensor_scalar_mul(
                    out=vext[:, t, h0:h0 + hn, 0:HD],
                    in0=pv[:, 0:hn * HD].rearrange("p (h d) -> p h d", d=HD),
                    scalar1=1.0 / QV,
                )

        def att_half(ps, n, hp, inline_v=False):
            """scores + exp + PV + normalize for head-pair hp, token half n."""
            qs = qkT[:, hp, :]
            ks = qkT[:, HP + hp, :]
            po = [
                ps.tile([P, 512], F32, tag="po", bufs=2, name=f"po_n{n}h{hp}s{s}")
                for s in range(2)
            ]
            for mt in range(NT):
                if inline_v and mt + 2 < NT:
                    # keep each vext tile ahead of its PV consumer in the
                    # PE queue (emission order IS the engine queue order)
                    emit_v(ps, mt + 2)
                psS = ps.tile([P, 2, 512], F32, tag="psS", bufs=2, name=f"psS_n{n}h{hp}m{mt}")
                for sub in range(2):
                    doff = sub * HD
                    nc.tensor.matmul(
                        psS[:, sub, :],
                        ks[doff:doff + HD, mt * P:(mt + 1) * P],
                        qs[doff:doff + HD, n * 512:(n + 1) * 512],
                        start=True,
                        stop=True,
                    )
                pp = work.tile([P, 2, 512], BF16, tag="ppair", bufs=10, name=f"pp_n{n}h{hp}m{mt}")
                nc.scalar.activation(out=pp, in_=psS, func=AF.Exp, scale=SCALE)
                for sub in range(2):
                    nc.tensor.matmul(
                        po[sub][0:HD + 1, :],
                        vext[:, mt, 2 * hp + sub, :],
                        pp[:, sub, :],
                        start=(mt == 0),
                        stop=(mt == NT - 1),
                    )
            # copy PSUM out fast (frees po for the next head-pair), then
            # batch the two denominator reciprocals off the critical path
            ous, dpack = [], stat_pool.tile([2, 512], F32, tag="dpack", bufs=2, name=f"dp_n{n}h{hp}")
            for sub in range(2):
                ou = work.tile([HD + 1, 512], F32, tag="ou", bufs=6, name=f"ou_n{n}h{hp}s{sub}")
                nc.vector.tensor_copy(out=ou, in_=po[sub][0:HD + 1, :])
                nc.sync.dma_start(out=dpack[sub:sub + 1, :], in_=ou[HD:HD + 1, :])
                ous.append(ou)
            rpack = stat_pool.tile([2, 512], F32, tag="rpack", bufs=2, name=f"rp_n{n}h{hp}")
            nc.vector.reciprocal_approx_fast(out=rpack, in_=dpack)
            for sub in range(2):
                doff = sub * HD
                rtmp = stat_pool.tile([1, 512], F32, tag="rtmp", bufs=2, name=f"rt_n{n}h{hp}s{sub}")
                nc.sync.dma_start(out=rtmp, in_=rpack[sub:sub + 1, :])
                rb = work.tile([HD, 512], F32, tag="rb", bufs=6, name=f"rb_n{n}h{hp}s{sub}")
                nc.gpsimd.partition_broadcast(rb, rtmp)
                nc.vector.tensor_tensor(
                    out=oT[doff:doff + HD, hp, n * 512:(n + 1) * 512],
                    in0=ous[sub][0:HD, :],
                    in1=rb,
                    op=ALU.mult,
                )

        def proj_ln2(ps, n):
            """proj (DR) + residual + LN2 + transpose to h2T for token half n."""
            # after attention half 1 the psS banks are free; using them for
            # the n=1 transposes decouples them from the pqk (ppr/acc) rotation
            trtag = "pqk" if n == 0 else "psS"
            for me in range(EC):
                ppr = ps.tile([P, 512], F32, tag="pqk", bufs=2, name=f"ppr_{me}_{n}")
                for i in range(ECP):
                    nc.tensor.matmul(
                        ppr,
                        pw_sb[:, 2 * i:2 * i + 2, me * P:(me + 1) * P],
                        oT[:, 2 * i:2 * i + 2, n * 512:(n + 1) * 512],
                        start=(i == 0),
                        stop=(i == ECP - 1),
                        perf_mode=DRM,
                    )
                prn = work.tile([P, 512], BF16, tag="prn", name=f"prn_{me}_{n}")
                nc.scalar.activation(
                    out=prn, in_=ppr, func=AF.Identity,
                    bias=pb_sb[:, me:me + 1], scale=1.0 / QP,
                )
                ptr = ps.tile([P, 8, P], BF16, tag=trtag, bufs=2, name=f"trp_{me}_{n}")
                for j in range(4):
                    nc.tensor.transpose(ptr[:, j, :], prn[:, j * P:(j + 1) * P], ident)
                nc.vector.tensor_tensor(
                    out=x1[:, 4 * n:4 * n + 4, me * P:(me + 1) * P],
                    in0=x1[:, 4 * n:4 * n + 4, me * P:(me + 1) * P],
                    in1=ptr[:, 0:4, :],
                    op=ALU.add,
                )
            mvs2, hs2 = [], []
            for j in range(4):
                t = 4 * n + j
                mv = stat_pool.tile([P, 2], F32, tag="mv", bufs=NT, name=f"mv2_{t}")
                stats = stat_pool.tile([P, 3, 6], F32, tag="stats", name=f"st2_{t}")
                _ln_stats(nc, x1[:, t, :], mv, stats, eps_t)
                mvs2.append(mv)
            for j in range(4):
                t = 4 * n + j
                h_t = work.tile([P, EMB], BF16, tag="h", bufs=4, name=f"h2_{t}")
                _ln_apply(nc, x1[:, t, :], h_t, mvs2[j])
                hs2.append(h_t)
            for j in range(4):
                _transpose_to_featmajor(nc, tc, ps, hs2[j], h2T, 4 * n + j, tag=trtag)

        def fc1_pair(ps, n, hcp):
            """fc1 (DR) + gelu for one hc pair, token half n -> a2g[:, hcp]."""
            for sub in range(2):
                hc = 2 * hcp + sub
                pf1 = ps.tile([P, 512], F32, tag="po", bufs=2, name=f"pf1_{n}_{hc}")
                splits = ((0, 512),)
                for c0, cw in splits:
                    for i in range(ECP):
                        nc.tensor.matmul(
                            pf1[:, c0:c0 + cw],
                            f1w_sb[:, 2 * i:2 * i + 2, hc * P:(hc + 1) * P],
                            h2T[:, 2 * i:2 * i + 2, n * 512 + c0:n * 512 + c0 + cw],
                            start=(i == 0),
                            stop=(i == ECP - 1),
                            perf_mode=DRM,
                        )
                nc.scalar.activation(
                    out=a2g[:, hcp, sub, :], in_=pf1, func=AF.Gelu,
                    bias=f1b_sb[:, hc:hc + 1], scale=1.0 / QF1,
                )

        def fc2_e(ps, n, e):
            """fc2 (DR) accumulation for output chunk e, token half n -> fr tile."""
            acc = ps.tile([P, 512], F32, tag="pqk", bufs=2, name=f"acc_{n}_{e}")
            for hcp in range(HCP):
                nc.tensor.matmul(
                    acc,
                    f2w_sb[:, 2 * hcp:2 * hcp + 2, e * P:(e + 1) * P],
                    a2g[:, hcp, :, :],
                    start=(hcp == 0),
                    stop=(hcp == HCP - 1),
                    perf_mode=DRM,
                )
            fr = work.tile([P, 512], BF16, tag="fr", bufs=8, name=f"fr_{n}_{e}")
            if n == 0:
                # window phase: ACT is exp-saturated, use DVE
                nc.vector.tensor_scalar(
                    out=fr, in0=acc,
                    scalar1=1.0 / QF2, scalar2=f2b_sb[:, e:e + 1],
                    op0=ALU.mult, op1=ALU.add,
                )
            else:
                nc.scalar.activation(
                    out=fr, in_=acc, func=AF.Identity,
                    bias=f2b_sb[:, e:e + 1], scale=1.0 / QF2,
                )
            return fr

        def mlp_finish(ps, n, frs):
            """transpose fc2 out + residual + DMA out for token half n."""
            if n == 0:
                for e in range(EC):
                    ptr = ps.tile([P, 8, P], BF16, tag="psS", bufs=2, name=f"trf_{n}_{e}")
                    for j in range(4):
                        nc.tensor.transpose(ptr[:, j, :], frs[e][:, j * P:(j + 1) * P], ident)
                    nc.vector.tensor_tensor(
                        out=x1[:, 4 * n:4 * n + 4, e * P:(e + 1) * P],
                        in0=x1[:, 4 * n:4 * n + 4, e * P:(e + 1) * P],
                        in1=ptr[:, 0:4, :],
                        op=ALU.add,
                    )
                for j in range(4):
                    t = 4 * n + j
                    nc.sync.dma_start(out=out_r[:, t, :], in_=x1[:, t, :])
            else:
                # tail half: go tile-pair-major so the first two output DMAs
                # launch while the second pair's residual adds still run
                for pair in range(2):
                    for e in range(EC):
                        ptr = ps.tile([P, 8, P], BF16, tag="psS", bufs=2,
                                      name=f"trf_{n}_{e}_p{pair}")
                        for j in range(2):
                            jj = 2 * pair + j
                            nc.tensor.transpose(
                                ptr[:, j, :], frs[e][:, jj * P:(jj + 1) * P], ident)
                        nc.vector.tensor_tensor(
                            out=x1[:, 4 * n + 2 * pair:4 * n + 2 * pair + 2, e * P:(e + 1) * P],
                            in0=x1[:, 4 * n + 2 * pair:4 * n + 2 * pair + 2, e * P:(e + 1) * P],
                            in1=ptr[:, 0:2, :],
                            op=ALU.add,
                        )
                    for j in range(2):
                        t = 4 * n + 2 * pair + j
                        nc.sync.dma_start(out=out_r[:, t, :], in_=x1[:, t, :])

        out_r = out_d.rearrange("(t p) e -> p t e", p=P)

        # ====== Main pipeline: one PSUM pool (psS 4 + po 2 + pqk 2 banks) so
        # ====== the scheduler can overlap phases freely (no pool barriers)
        with tc.tile_pool(name="psM", space="PSUM", bufs=1) as psM:
            nc.vector.memset(vext[:, :, :, HD:HD + 1], 1.0)
            # attention half 0: scores/exp for head-pair 0 are emitted before
            # the v projections so the ACT exp stream starts as early as
            # possible (the scheduler slots v in before the PV consumers)
            emit_qk(psM, 0)
            emit_v(psM, 0)
            emit_v(psM, 1)
            att_half(psM, 0, 0, inline_v=True)
            emit_qk(psM, 1)
            # head-pair 0 of half 1 runs here: this phase is PE-bound, so the
            # extra exp stream fills ACT slack and shrinks the later window
            att_half(psM, 1, 0)
            for hp in range(1, HP):
                if hp + 1 < HP:
                    emit_qk(psM, hp + 1)
                att_half(psM, 0, hp)
            # proj + LN2 half 0; attention half 1 exp can flow during this
            proj_ln2(psM, 0)
            # window: attention half 1 (pairs 1..5) || fc1 half 0 front-loaded
            # so fc2 half 0 can interleave with the last two head-pairs' exps
            pairs = [4, 4, 4, 0, 0]
            done = 0
            frs0 = []
            for i in range(1, HP):
                att_half(psM, 1, i)
                for k in range(pairs[i - 1]):
                    fc1_pair(psM, 0, done + k)
                done += pairs[i - 1]
                if i == 4:
                    frs0 += [fc2_e(psM, 0, e) for e in range(3)]
            frs0 += [fc2_e(psM, 0, e) for e in range(3, EC)]
            # finish(0) first: its transposes fill the PE hole while the last
            # head-pair's normalize tail gates proj(1) (psS tag, no pqk clash)
            mlp_finish(psM, 0, frs0)
            proj_ln2(psM, 1)
            for hcp in range(HCP):
                fc1_pair(psM, 1, hcp)
            frs1 = [fc2_e(psM, 1, e) for e in range(EC)]
            mlp_finish(psM, 1, frs1)


def fold_inputs(inputs):
    """Fold LN gamma/beta and v-bias into downstream weights (exact math)."""
    f = {k: np.asarray(v, dtype=np.float32) for k, v in inputs.items()}
    qkw = f["ln1_g"][:, None] * f["qk_w"]
    qkb = f["ln1_b"] @ f["qk_w"]
    vw = f["ln1_g"][:, None] * f["v_w"]
    vb = f["ln1_b"] @ f["v_w"]
    # softmax rows sum to 1 => o = attn @ (v + 1 vb^T) = attn@v + vb
    pb = f["proj_b"] + vb @ f["proj_w"]
    f1w = f["ln2_g"][:, None] * f["fc1_w"]
    f1b = f["fc1_b"] + f["ln2_b"] @ f["fc1_w"]
    import ml_dtypes

    fp8 = ml_dtypes.float8_e4m3fn

    def q8(w, s):
        return np.ascontiguousarray(np.clip(w * s, -240.0, 240.0).astype(fp8))

    return {
        "qkw": q8(qkw, QQK),
        "qkb": np.ascontiguousarray(qkb),
        "vw": q8(vw, QV),
        "pw": q8(f["proj_w"], QP),
        "pb": np.ascontiguousarray(pb),
        "f1w": q8(f1w, QF1),
        "f1b": np.ascontiguousarray(f1b),
        "f2w": q8(f["fc2_w"], QF2),
        "f2b": np.ascontiguousarray(f["fc2_b"]),
    }


_INPUT_SHAPES = {
    "x": (SEQ, EMB),
    "qkw": (EMB, 2 * EMB),
    "qkb": (2 * EMB,),
    "vw": (EMB, EMB),
    "pw": (EMB, EMB),
    "pb": (EMB,),
    "f1w": (EMB, MLPD),
    "f1b": (MLPD,),
    "f2w": (MLPD, EMB),
    "f2b": (EMB,),
}

_N_CORES = 8
_compiled = {}


def _build_nc(num_devices=_N_CORES):
    import concourse.tile as tile
    from concourse import bacc

    nc = bacc.Bacc(
        "TRN2", target_bir_lowering=False, debug=False, num_devices=num_devices
    )
    _FP8_INPUTS = {"qkw", "vw", "pw", "f1w", "f2w"}
    ins = {
        name: nc.dram_tensor(
            name, list(shape), F8 if name in _FP8_INPUTS else F32,
            kind="ExternalInput",
        ).ap()
        for name, shape in _INPUT_SHAPES.items()
    }
    out = nc.dram_tensor("out", [SEQ, EMB], F32, kind="ExternalOutput").ap()
    with tile.TileContext(nc) as tc:
        build_block(tc, {"out": out}, ins)
    nc.compile()
    return nc


def kernel(**inputs):
    """Full-input entry point: x [8, 1024, 768] + weights -> [8, 1024, 768]."""
    from concourse.bass_utils import run_bass_kernel_spmd

    if "nc" not in _compiled:
        _compiled["nc"] = _build_nc()
    nc = _compiled["nc"]

    x = np.asarray(inputs["x"], dtype=np.float32)
    folded = fold_inputs({k: v for k, v in inputs.items() if k != "x"})
    in_maps = [
        {"x": np.ascontiguousarray(x[c]), **folded} for c in range(_N_CORES)
    ]
    res = run_bass_kernel_spmd(nc, in_maps, core_ids=list(range(_N_CORES)))
    return np.stack([res.results[c]["out"] for c in range(_N_CORES)]).astype(
        np.float32
    )


# revision 79
# speedup vs baseline: 1.0081x; 1.0081x over previous
"""Transformer block kernel for TRN2 (Bass/Tile), one batch element per core.

Computes (per core, x [1024, 768] f32):
    h  = LN(x) (gamma/beta pre-folded into weights on host), stored fp8
    qk = h @ qkw + qkb (fp8 DoubleRow, K=256/mm); q,k fp8 head-major
    v  = h @ vw (fp8 DR), packed into vext with a ones column (bf16)
    S^T[m,n] = (k_m . q_n) / 8 for two heads concurrently (row-split PE)
    P = exp(S^T) bf16; oe = [v;1]^T @ P -> rows 0..63 o^T, row 64 denom
    o^T = oe / denom -> fp8;  x1 = x + o @ pw + pb (fp8 DR)
    h2 = LN2(x1) -> fp8; out = x1 + gelu(h2 @ f1w + f1b) @ f2w + f2b (fp8 DR)

Two-half pipeline: attention exp for tokens 512..1023 (ACT engine) overlaps
the fc2 matmuls for tokens 0..511 (PE), since exp is the attention bottleneck.

fp8 weights are pre-scaled by powers of 2 on host; the inverse scale is folded
into the PSUM->SBUF copy (DVE tensor_scalar or ACT activation scale).
"""

import sys
from contextlib import ExitStack

if "/opt/trn_rl_repo" not in sys.path:
    sys.path.insert(0, "/opt/trn_rl_repo")

import numpy as np

import concourse.bass as bass
import concourse.mybir as mybir
from concourse.masks import make_identity

F32 = mybir.dt.float32
BF16 = mybir.dt.bfloat16
F8 = mybir.dt.float8e4
DRM = mybir.MatmulPerfMode.DoubleRow
AF = mybir.ActivationFunctionType
ALU = mybir.AluOpType

P = 128
EMB = 768
SEQ = 1024
NH = 12
HD = 64
MLPD = 3072
EC = EMB // P      # 6 embedding chunks
ECP = EC // 2      # 3 DR k-pair chunks
NT = SEQ // P      # 8 token tiles
NC2 = SEQ // 512   # 2 token halves
HC = MLPD // P     # 24 hidden chunks
HCP = HC // 2      # 12 hidden pairs
HP = NH // 2       # 6 head pairs
EPS = 1e-5
SCALE = HD ** -0.5

# fp8 weight pre-scales (powers of two; inverse folded into PSUM copy-out)
QQK = 4096.0
QV = 4096.0
QP = 4096.0
QF1 = 4096.0
QF2 = 8192.0


def _ln_stats(nc, x_ap, mv, stats, eps_t):
    """bn stats + rstd for one [128, EMB] tile; mv = [mean, rstd]."""
    xg = x_ap.rearrange("p (g d) -> p g d", d=256)
    for g in range(3):
        nc.vector.bn_stats(out=stats[:, g, :], in_=xg[:, g, :])
    nc.vector.bn_aggr(out=mv, in_=stats)
    nc.scalar.activation(out=mv[:, 1:2], in_=mv[:, 1:2], func=AF.Sqrt, bias=eps_t, scale=1.0)
    nc.vector.reciprocal(out=mv[:, 1:2], in_=mv[:, 1:2])


def _ln_apply(nc, x_ap, h_out, mv, negmr=None):
    if negmr is not None:
        # ACT variant for the head phase (DVE is congested, ACT idle):
        # h = x*rstd + (-mean*rstd)
        nc.scalar.activation(
            out=h_out, in_=x_ap, func=AF.Identity,
            scale=mv[:, 1:2], bias=negmr,
        )
    else:
        nc.vector.tensor_scalar(
            out=h_out,
            in0=x_ap,
            scalar1=mv[:, 0:1],
            scalar2=mv[:, 1:2],
            op0=ALU.subtract,
            op1=ALU.mult,
        )


def _transpose_to_featmajor(nc, tc, pool_ps, src_tok, dstT, t, tag="tr", copy_dve=False):
    """PE-transpose token-major src_tok [128, EMB] into dstT [:, e, t*128:(t+1)*128]."""
    ident = tc._block_ident
    for group_start, group_n in ((0, 4), (4, 2)):
        # [P, 8*P] bf16 = 2KB/partition so the tile is size-compatible with
        # the shared "pqk" psum tag; only the first 4*P columns are used.
        ptr = pool_ps.tile([P, 8 * P], BF16, tag=tag, bufs=2, name=f"ptr_t{t}_{group_start}_{dstT.tensor.name}")
        for j in range(group_n):
            e = group_start + j
            nc.tensor.transpose(
                ptr[:, j * P:(j + 1) * P],
                src_tok[:, e * P:(e + 1) * P],
                ident,
            )
        nc.scalar.copy(
            out=dstT[:, group_start:group_start + group_n, t * P:(t + 1) * P],
            in_=ptr[:, :group_n * P].rearrange("p (j q) -> p j q", q=P),
        )


def build_block(tc, outs, ins):
    nc = tc.nc
    x_d = ins["x"]
    qkw_d, qkb_d = ins["qkw"], ins["qkb"]
    vw_d = ins["vw"]
    pw_d, pb_d = ins["pw"], ins["pb"]
    f1w_d, f1b_d = ins["f1w"], ins["f1b"]
    f2w_d, f2b_d = ins["f2w"], ins["f2b"]
    out_d = outs["out"]

    with ExitStack() as ctx:
        consts = ctx.enter_context(tc.tile_pool(name="consts", bufs=1))
        ident = consts.tile([P, P], BF16)
        make_identity(nc, ident)
        tc._block_ident = ident
        eps_t = consts.tile([P, 1], F32)
        nc.vector.memset(eps_t, EPS)
        qkb_sb = consts.tile([P, 2 * EC], F32)
        pb_sb = consts.tile([P, EC], F32)
        f1b_sb = consts.tile([P, HC], F32)
        f2b_sb = consts.tile([P, EC], F32)

        # Persistent SBUF tensors
        glob = ctx.enter_context(tc.tile_pool(name="glob", bufs=1))
        x1 = glob.tile([P, NT, EMB], F32)             # residual stream
        actT = glob.tile([P, EC, SEQ], F8)            # LN1 out, feature-major
        h2T = glob.tile([P, EC, SEQ], F8)             # LN2 out, feature-major
        oT = glob.tile([P, EC, SEQ], F8)              # attention out, feature-major
        qkT = glob.tile([P, 2 * HP, SEQ], F8)         # q (0..5) and k (6..11) blocks
        vext = glob.tile([P, NT, NH, HD + 1], BF16)   # v + ones column
        a2g = glob.tile([P, HCP, 2, 512], F8)         # gelu out, hc-paired
        qkw_sb = glob.tile([P, EC, 2 * EMB], F8)
        vw_sb = glob.tile([P, EC, EMB], F8)
        pw_sb = glob.tile([P, EC, EMB], F8)
        f1w_sb = glob.tile([P, EC, MLPD], F8)
        f2w_sb = glob.tile([P, HC, EMB], F8)

        work = ctx.enter_context(tc.tile_pool(name="work", bufs=3))
        stat_pool = ctx.enter_context(tc.tile_pool(name="stat", bufs=4))

        # ---- DMAs: x first (blocks everything), then weights in use order ----
        # (spreading these across other engine queues was tried and measured
        # slower: scalar-queue copies delay the ACT stream, gpsimd DGE is slow)
        x_r = x_d.rearrange("(t p) e -> p t e", p=P)
        for t in range(NT):
            nc.sync.dma_start(out=x1[:, t, :], in_=x_r[:, t, :])
        nc.sync.dma_start(out=qkw_sb, in_=qkw_d.rearrange("(kc p) o -> p kc o", p=P))
        nc.sync.dma_start(out=vw_sb, in_=vw_d.rearrange("(kc p) o -> p kc o", p=P))
        nc.sync.dma_start(out=qkb_sb, in_=qkb_d.rearrange("(m p) -> p m", p=P))
        nc.sync.dma_start(out=pb_sb, in_=pb_d.rearrange("(m p) -> p m", p=P))
        nc.sync.dma_start(out=f1b_sb, in_=f1b_d.rearrange("(m p) -> p m", p=P))
        nc.sync.dma_start(out=f2b_sb, in_=f2b_d.rearrange("(m p) -> p m", p=P))
        nc.sync.dma_start(out=pw_sb, in_=pw_d.rearrange("(kc p) e -> p kc e", p=P))
        nc.sync.dma_start(out=f1w_sb, in_=f1w_d.rearrange("(kc p) o -> p kc o", p=P))
        nc.sync.dma_start(out=f2w_sb, in_=f2w_d.rearrange("(hc p) e -> p hc e", p=P))

        # ================= Phase A: LN1 + transpose to actT =================
        with tc.tile_pool(name="psA", space="PSUM", bufs=2) as psA:
            hs, mvs = [], []
            for t in range(NT):
                mv = stat_pool.tile([P, 2], F32, tag="mv", bufs=NT, name=f"mv1_{t}")
                stats = stat_pool.tile([P, 3, 6], F32, tag="stats", name=f"st1_{t}")
                _ln_stats(nc, x1[:, t, :], mv, stats, eps_t)
                mvs.append(mv)
            for t in range(NT):
                h_t = work.tile([P, EMB], BF16, tag="h", bufs=4, name=f"h_{t}")
                _ln_apply(nc, x1[:, t, :], h_t, mvs[t])
                hs.append(h_t)
            for t in range(NT):
                _transpose_to_featmajor(nc, tc, psA, hs[t], actT, t)

        # ---------- shared emitters ----------
        def emit_qk(ps, hp):
            """qk projection (DR) for head-pair hp -> qkT blocks hp and HP+hp."""
            for m in (hp, HP + hp):
                for nn in range(NC2):
                    pqk = ps.tile([P, 512], F32, tag="pqk", bufs=2, name=f"pqk_{m}_{nn}")
                    for i in range(ECP):
                        nc.tensor.matmul(
                            pqk,
                            qkw_sb[:, 2 * i:2 * i + 2, m * P:(m + 1) * P],
                            actT[:, 2 * i:2 * i + 2, nn * 512:(nn + 1) * 512],
                            start=(i == 0),
                            stop=(i == ECP - 1),
                            perf_mode=DRM,
                        )
                    nc.vector.tensor_scalar(
                        out=qkT[:, m, nn * 512:(nn + 1) * 512],
                        in0=pqk,
                        scalar1=1.0 / QQK,
                        scalar2=qkb_sb[:, m:m + 1],
                        op0=ALU.mult,
                        op1=ALU.add,
                    )

        def emit_v(ps, t):
            """v projection (DR) for token tile t -> vext[:, t, :, 0:HD]."""
            for half, (c0, cw, h0, hn) in ((0, (0, 512, 0, 8)), (1, (512, 256, 8, 4))):
                pv = ps.tile([P, 512], F32, tag="pqk", bufs=2, name=f"pv_{t}_{half}")
                for i in range(ECP):
                    nc.tensor.matmul(
                        pv[:, :cw],
                        actT[:, 2 * i:2 * i + 2, t * P:(t + 1) * P],
                        vw_sb[:, 2 * i:2 * i + 2, c0:c0 + cw],
                        start=(i == 0),
                        stop=(i == ECP - 1),
                        perf_mode=DRM,
                    )
                nc.vector.tensor_scalar_mul(
                    out=vext[:, t, h0:h0 + hn, 0:HD],
                    in0=pv[:, 0:hn * HD].rearrange("p (h d) -> p h d", d=HD),
                    scalar1=1.0 / QV,
                )

        def att_half(ps, n, hp, inline_v=False):
            """scores + exp + PV + normalize for head-pair hp, token half n."""
            qs = qkT[:, hp, :]
            ks = qkT[:, HP + hp, :]
            po = [
                ps.tile([P, 512], F32, tag="po", bufs=2, name=f"po_n{n}h{hp}s{s}")
                for s in range(2)
            ]
            for mt in range(NT):
                if inline_v and mt + 2 < NT:
                    # keep each vext tile ahead of its PV consumer in the
                    # PE queue (emission order IS the engine queue order)
                    emit_v(ps, mt + 2)
                psS = ps.tile([P, 2, 512], F32, tag="psS", bufs=2, name=f"psS_n{n}h{hp}m{mt}")
                for sub in range(2):
                    doff = sub * HD
                    nc.tensor.matmul(
                        psS[:, sub, :],
                        ks[doff:doff + HD, mt * P:(mt + 1) * P],
                        qs[doff:doff + HD, n * 512:(n + 1) * 512],
                        start=True,
                        stop=True,
                    )
                pp = work.tile([P, 2, 512], BF16, tag="ppair", bufs=10, name=f"pp_n{n}h{hp}m{mt}")
                nc.scalar.activation(out=pp, in_=psS, func=AF.Exp, scale=SCALE)
                for sub in range(2):
                    nc.tensor.matmul(
                        po[sub][0:HD + 1, :],
                        vext[:, mt, 2 * hp + sub, :],
                        pp[:, sub, :],
                        start=(mt == 0),
                        stop=(mt == NT - 1),
                    )
            # copy PSUM out fast (frees po for the next head-pair), then
            # batch the two denominator reciprocals off the critical path
            ous, dpack = [], stat_pool.tile([2, 512], F32, tag="dpack", bufs=2, name=f"dp_n{n}h{hp}")
            for sub in range(2):
                ou = work.tile([HD + 1, 512], F32, tag="ou", bufs=6, name=f"ou_n{n}h{hp}s{sub}")
                nc.vector.tensor_copy(out=ou, in_=po[sub][0:HD + 1, :])
                nc.sync.dma_start(out=dpack[sub:sub + 1, :], in_=ou[HD:HD + 1, :])
                ous.append(ou)
            rpack = stat_pool.tile([2, 512], F32, tag="rpack", bufs=2, name=f"rp_n{n}h{hp}")
            nc.vector.reciprocal_approx_fast(out=rpack, in_=dpack)
            for sub in range(2):
                doff = sub * HD
                rtmp = stat_pool.tile([1, 512], F32, tag="rtmp", bufs=2, name=f"rt_n{n}h{hp}s{sub}")
                nc.sync.dma_start(out=rtmp, in_=rpack[sub:sub + 1, :])
                rb = work.tile([HD, 512], F32, tag="rb", bufs=6, name=f"rb_n{n}h{hp}s{sub}")
                nc.gpsimd.partition_broadcast(rb, rtmp)
                nc.vector.tensor_tensor(
                    out=oT[doff:doff + HD, hp, n * 512:(n + 1) * 512],
                    in0=ous[sub][0:HD, :],
                    in1=rb,
                    op=ALU.mult,
                )

        def proj_ln2(ps, n):
            """proj (DR) + residual + LN2 + transpose to h2T for token half n."""
            # after attention half 1 the psS banks are free; using them for
            # the n=1 transposes decouples them from the pqk (ppr/acc) rotation
            trtag = "pqk" if n == 0 else "psS"
            for me in range(EC):
                ppr = ps.tile([P, 512], F32, tag="pqk", bufs=2, name=f"ppr_{me}_{n}")
                for i in range(ECP):
                    nc.tensor.matmul(
                        ppr,
                        pw_sb[:, 2 * i:2 * i + 2, me * P:(me + 1) * P],
                        oT[:, 2 * i:2 * i + 2, n * 512:(n + 1) * 512],
                        start=(i == 0),
                        stop=(i == ECP - 1),
                        perf_mode=DRM,
                    )
                prn = work.tile([P, 512], BF16, tag="prn", name=f"prn_{me}_{n}")
                nc.scalar.activation(
                    out=prn, in_=ppr, func=AF.Identity,
                    bias=pb_sb[:, me:me + 1], scale=1.0 / QP,
                )
                ptr = ps.tile([P, 8, P], BF16, tag=trtag, bufs=2, name=f"trp_{me}_{n}")
                for j in range(4):
                    nc.tensor.transpose(ptr[:, j, :], prn[:, j * P:(j + 1) * P], ident)
                nc.vector.tensor_tensor(
                    out=x1[:, 4 * n:4 * n + 4, me * P:(me + 1) * P],
                    in0=x1[:, 4 * n:4 * n + 4, me * P:(me + 1) * P],
                    in1=ptr[:, 0:4, :],
                    op=ALU.add,
                )
            mvs2, hs2 = [], []
            for j in range(4):
                t = 4 * n + j
                mv = stat_pool.tile([P, 2], F32, tag="mv", bufs=NT, name=f"mv2_{t}")
                stats = stat_pool.tile([P, 3, 6], F32, tag="stats", name=f"st2_{t}")
                _ln_stats(nc, x1[:, t, :], mv, stats, eps_t)
                mvs2.append(mv)
            for j in range(4):
                t = 4 * n + j
                h_t = work.tile([P, EMB], BF16, tag="h", bufs=4, name=f"h2_{t}")
                _ln_apply(nc, x1[:, t, :], h_t, mvs2[j])
                hs2.append(h_t)
            for j in range(4):
                _transpose_to_featmajor(nc, tc, ps, hs2[j], h2T, 4 * n + j, tag=trtag)

        def fc1_pair(ps, n, hcp):
            """fc1 (DR) + gelu for one hc pair, token half n -> a2g[:, hcp]."""
            for sub in range(2):
                hc = 2 * hcp + sub
                pf1 = ps.tile([P, 512], F32, tag="po", bufs=2, name=f"pf1_{n}_{hc}")
                splits = ((0, 512),)
                for c0, cw in splits:
                    for i in range(ECP):
                        nc.tensor.matmul(
                            pf1[:, c0:c0 + cw],
                            f1w_sb[:, 2 * i:2 * i + 2, hc * P:(hc + 1) * P],
                            h2T[:, 2 * i:2 * i + 2, n * 512 + c0:n * 512 + c0 + cw],
                            start=(i == 0),
                            stop=(i == ECP - 1),
                            perf_mode=DRM,
                        )
                nc.scalar.activation(
                    out=a2g[:, hcp, sub, :], in_=pf1, func=AF.Gelu,
                    bias=f1b_sb[:, hc:hc + 1], scale=1.0 / QF1,
                )

        def fc2_e(ps, n, e):
            """fc2 (DR) accumulation for output chunk e, token half n -> fr tile."""
            acc = ps.tile([P, 512], F32, tag="pqk", bufs=2, name=f"acc_{n}_{e}")
            for hcp in range(HCP):
                nc.tensor.matmul(
                    acc,
                    f2w_sb[:, 2 * hcp:2 * hcp + 2, e * P:(e + 1) * P],
                    a2g[:, hcp, :, :],
                    start=(hcp == 0),
                    stop=(hcp == HCP - 1),
                    perf_mode=DRM,
                )
            fr = work.tile([P, 512], BF16, tag="fr", bufs=8, name=f"fr_{n}_{e}")
            if n == 0:
                # window phase: ACT is exp-saturated, use DVE
                nc.vector.tensor_scalar(
                    out=fr, in0=acc,
                    scalar1=1.0 / QF2, scalar2=f2b_sb[:, e:e + 1],
                    op0=ALU.mult, op1=ALU.add,
                )
            else:
                nc.scalar.activation(
                    out=fr, in_=acc, func=AF.Identity,
                    bias=f2b_sb[:, e:e + 1], scale=1.0 / QF2,
                )
            return fr

        def mlp_finish(ps, n, frs):
            """transpose fc2 out + residual + DMA out for token half n."""
            if n == 0:
                for e in range(EC):
                    ptr = ps.tile([P, 8, P], BF16, tag="psS", bufs=2, name=f"trf_{n}_{e}")
                    for j in range(4):
                        nc.tensor.transpose(ptr[:, j, :], frs[e][:, j * P:(j + 1) * P], ident)
                    nc.vector.tensor_tensor(
                        out=x1[:, 4 * n:4 * n + 4, e * P:(e + 1) * P],
                        in0=x1[:, 4 * n:4 * n + 4, e * P:(e + 1) * P],
                        in1=ptr[:, 0:4, :],
                        op=ALU.add,
                    )
                for j in range(4):
                    t = 4 * n + j
                    nc.sync.dma_start(out=out_r[:, t, :], in_=x1[:, t, :])
            else:
                # tail half: go tile-pair-major so the first two output DMAs
                # launch while the second pair's residual adds still run
                for pair in range(2):
                    for e in range(EC):
                        ptr = ps.tile([P, 8, P], BF16, tag="psS", bufs=2,
                                      name=f"trf_{n}_{e}_p{pair}")
                        for j in range(2):
                            jj = 2 * pair + j
                            nc.tensor.transpose(
                                ptr[:, j, :], frs[e][:, jj * P:(jj + 1) * P], ident)
                        nc.vector.tensor_tensor(
                            out=x1[:, 4 * n + 2 * pair:4 * n + 2 * pair + 2, e * P:(e + 1) * P],
                            in0=x1[:, 4 * n + 2 * pair:4 * n + 2 * pair + 2, e * P:(e + 1) * P],
                            in1=ptr[:, 0:2, :],
                            op=ALU.add,
                        )
                    for j in range(2):
                        t = 4 * n + 2 * pair + j
                        nc.sync.dma_start(out=out_r[:, t, :], in_=x1[:, t, :])

        out_r = out_d.rearrange("(t p) e -> p t e", p=P)

        # ====== Main pipeline: one PSUM pool (psS 4 + po 2 + pqk 2 banks) so
        # ====== the scheduler can overlap phases freely (no pool barriers)
        with tc.tile_pool(name="psM", space="PSUM", bufs=1) as psM:
            nc.vector.memset(vext[:, :, :, HD:HD + 1], 1.0)
            # attention half 0: scores/exp for head-pair 0 are emitted before
            # the v projections so the ACT exp stream starts as early as
            # possible (the scheduler slots v in before the PV consumers)
            emit_qk(psM, 0)
            emit_v(psM, 0)
            emit_v(psM, 1)
            att_half(psM, 0, 0, inline_v=True)
            emit_qk(psM, 1)
            # head-pair 0 of half 1 runs here: this phase is PE-bound, so the
            # extra exp stream fills ACT slack and shrinks the later window
            att_half(psM, 1, 0)
            for hp in range(1, HP):
                if hp + 1 < HP:
                    emit_qk(psM, hp + 1)
                att_half(psM, 0, hp)
            # proj + LN2 half 0; attention half 1 exp can flow during this
            proj_ln2(psM, 0)
            # window: attention half 1 (pairs 1..5) || fc1 half 0 front-loaded
            # so fc2 half 0 can interleave with the last two head-pairs' exps
            pairs = [4, 4, 4, 0, 0]
            done = 0
            frs0 = []
            for i in range(1, HP):
                att_half(psM, 1, i)
                for k in range(pairs[i - 1]):
                    fc1_pair(psM, 0, done + k)
                done += pairs[i - 1]
                if i == 4:
                    frs0 += [fc2_e(psM, 0, e) for e in range(3)]
            frs0 += [fc2_e(psM, 0, e) for e in range(3, EC)]
            proj_ln2(psM, 1)
            mlp_finish(psM, 0, frs0)
            for hcp in range(HCP):
                fc1_pair(psM, 1, hcp)
            frs1 = [fc2_e(psM, 1, e) for e in range(EC)]
            mlp_finish(psM, 1, frs1)


def fold_inputs(inputs):
    """Fold LN gamma/beta and v-bias into downstream weights (exact math)."""
    f = {k: np.asarray(v, dtype=np.float32) for k, v in inputs.items()}
    qkw = f["ln1_g"][:, None] * f["qk_w"]
    qkb = f["ln1_b"] @ f["qk_w"]
    vw = f["ln1_g"][:, None] * f["v_w"]
    vb = f["ln1_b"] @ f["v_w"]
    # softmax rows sum to 1 => o = attn @ (v + 1 vb^T) = attn@v + vb
    pb = f["proj_b"] + vb @ f["proj_w"]
    f1w = f["ln2_g"][:, None] * f["fc1_w"]
    f1b = f["fc1_b"] + f["ln2_b"] @ f["fc1_w"]
    import ml_dtypes

    fp8 = ml_dtypes.float8_e4m3fn

    def q8(w, s):
        return np.ascontiguousarray(np.clip(w * s, -240.0, 240.0).astype(fp8))

    return {
        "qkw": q8(qkw, QQK),
        "qkb": np.ascontiguousarray(qkb),
        "vw": q8(vw, QV),
        "pw": q8(f["proj_w"], QP),
        "pb": np.ascontiguousarray(pb),
        "f1w": q8(f1w, QF1),
        "f1b": np.ascontiguousarray(f1b),
        "f2w": q8(f["fc2_w"], QF2),
        "f2b": np.ascontiguousarray(f["fc2_b"]),
    }


_INPUT_SHAPES = {
    "x": (SEQ, EMB),
    "qkw": (EMB, 2 * EMB),
    "qkb": (2 * EMB,),
    "vw": (EMB, EMB),
    "pw": (EMB, EMB),
    "pb": (EMB,),
    "f1w": (EMB, MLPD),
    "f1b": (MLPD,),
    "f2w": (MLPD, EMB),
    "f2b": (EMB,),
}

_N_CORES = 8
_compiled = {}


def _build_nc(num_devices=_N_CORES):
    import concourse.tile as tile
    from concourse import bacc

    nc = bacc.Bacc(
        "TRN2", target_bir_lowering=False, debug=False, num_devices=num_devices
    )
    _FP8_INPUTS = {"qkw", "vw", "pw", "f1w", "f2w"}
    ins = {
        name: nc.dram_tensor(
            name, list(shape), F8 if name in _FP8_INPUTS else F32,
            kind="ExternalInput",
        ).ap()
        for name, shape in _INPUT_SHAPES.items()
    }
    out = nc.dram_tensor("out", [SEQ, EMB], F32, kind="ExternalOutput").ap()
    with tile.TileContext(nc) as tc:
        build_block(tc, {"out": out}, ins)
    nc.compile()
    return nc


def kernel(**inputs):
    """Full-input entry point: x [8, 1024, 768] + weights -> [8, 1024, 768]."""
    from concourse.bass_utils import run_bass_kernel_spmd

    if "nc" not in _compiled:
        _compiled["nc"] = _build_nc()
    nc = _compiled["nc"]

    x = np.asarray(inputs["x"], dtype=np.float32)
    folded = fold_inputs({k: v for k, v in inputs.items() if k != "x"})
    in_maps = [
        {"x": np.ascontiguousarray(x[c]), **folded} for c in range(_N_CORES)
    ]
    res = run_bass_kernel_spmd(nc, in_maps, core_ids=list(range(_N_CORES)))
    return np.stack([res.results[c]["out"] for c in range(_N_CORES)]).astype(
        np.float32
    )
